# revision 5
# baseline (speedup 1.0000x reference)
"""Trainium2 Bass kernel for nn_AMTCL_77867757077077 (AMTCL triplet-center loss).

dist[i,j] depends on j only through targets[j] (C=100 distinct columns):
    d2[k,i] = q[k] - 2*(u[k]@x_i) + (v[k]@x2_i),  v=2^w, u=v*c, q=sum v*c^2
    dist_ap[i] = sqrt(d2[t_i, i])
    dist_an[i] = min_{k != t_i, k present} sqrt(d2[k,i])
    per_sample = ap + relu(cc_i - an) ,  cc_i = centers_dist[t_i]

Key restructure vs v1 (which sqrt'ed the whole [100,512] matrix):
  min and sqrt commute, and relu(cc-an) = cc - min(an, cc).  So per core:
    m2[i]  = min_k( d2 + 1e9*onehot  ; row C = cc^2 )   (an^2 vs cc^2 merged)
    ap2[i] = min_k( d2 + 1e9*(1-onehot) )               (self row survives)
    partial = sum_i ( sqrt(ap2) - sqrt(m2) ) / B
  loss = sum_cores partial + sum_i cc_i / B   (host-side constant)
  Only ONE [128,8] sqrt on device; everything else stays in the squared
  domain (bf16).  qpen = q + 1e12*absent rides inside the masks.

Device pipeline per core (512 rows):
  - DMAs: trow [1,512] (targets, bf16) + biga [128,2048] (fp8 SwInterleave
    weights | qpen f32 | x-pair0) on sync; bigb [128,2048] (pair1|pair2) on
    scalar; cc2row [1,512] -> row 100 of the an-matrix.
  - PE warmup: zero-weight bf16 matmuls into the same PSUM accumulation
    group during the DMA wait (HAM un-throttles, real MMs run at 2.4GHz).
  - 3 dual-row fp8 matmuls (DoubleRowSwInterleave) -> s_ps = d2 - q.
  - gpsimd partition_broadcast of trow; DVE builds maskM = 1e9*oh + qpen,
    maskP = 1e9*(1-oh) + qpen (hidden under the DMA wait).
  - DVE: copy s_ps->bf16 halves, STT add masks -> sbigM/sbigP.
  - PE: 8 transposes into two PSUM [128,4,101] banks; DVE: 2 merged
    min-reduces -> mn8 [128,8] f32; ACT sqrt [128,8] (table load hidden
    early via a warmup sqrt); two K=128 N=4 matmuls with +-1/B weights;
    DVE 4-elem add-reduce; DMA the f32 scalar out.
"""

import numpy as np

NUM_CORES = 8
B = 4096
D = 384
C = 100
BL = B // NUM_CORES  # 512 rows per core
P = 128
NT = BL // P         # 4 row tiles per core
KD = D // P          # 3 contraction chunks

_CACHE = {}


def _build_nc():
    import concourse.bass as bass
    import concourse.bass_isa as bass_isa
    import concourse.bacc as bacc
    import concourse.tile as tile
    from concourse import mybir
    from concourse.masks import make_identity
    from contextlib import ExitStack

    f32 = mybir.dt.float32
    bf16 = mybir.dt.bfloat16
    fp8 = mybir.dt.float8e4

    nc = bacc.Bacc(
        "TRN2",
        target_bir_lowering=False,
        debug=False,
        enable_asserts=False,
        num_devices=NUM_CORES,
    )

    AW = 2048  # weights 768 | qpen 4 | pair0 1024 | pad
    BW = 4 * BL
    biga_ext = nc.dram_tensor("biga", [P, AW], mybir.dt.uint8, kind="ExternalInput").ap()
    bigb_ext = nc.dram_tensor("bigb", [P, BW], mybir.dt.uint8, kind="ExternalInput").ap()
    trow_ext = nc.dram_tensor("trow", [1, BL], bf16, kind="ExternalInput").ap()
    cc2_ext = nc.dram_tensor("cc2", [1, BL], bf16, kind="ExternalInput").ap()
    out_ext = nc.dram_tensor("out", [1, 1], f32, kind="ExternalOutput").ap()

    with tile.TileContext(nc) as tc, ExitStack() as ctx:
        singles = ctx.enter_context(tc.tile_pool(name="singles", bufs=1))
        ps_big = ctx.enter_context(tc.tile_pool(name="psbig", bufs=1, space="PSUM"))
        ps_trm = ctx.enter_context(tc.tile_pool(name="pstrm", bufs=1, space="PSUM"))
        ps_trp = ctx.enter_context(tc.tile_pool(name="pstrp", bufs=1, space="PSUM"))
        ps_misc = ctx.enter_context(tc.tile_pool(name="psmisc", bufs=1, space="PSUM"))

        # ---- input DMAs first: trow (tiny, needed early by the broadcast),
        # then the two big streams, then cc2 straight into the an-matrix row.
        trow = singles.tile([1, BL], bf16)
        nc.sync.dma_start(trow, trow_ext)
        biga = singles.tile([P, AW], mybir.dt.uint8)
        nc.sync.dma_start(biga, biga_ext)
        bigb = singles.tile([P, BW], mybir.dt.uint8)
        nc.scalar.dma_start(bigb, bigb_ext)
        sbigM = singles.tile([C + 1, BL], bf16)   # d2+1e9*oh+qpen; row C = cc^2
        nc.sync.dma_start(sbigM[C : C + 1, :], cc2_ext)

        # ---- gpsimd constants (hidden under the DMA wait)
        ident_bf = singles.tile([P, P], bf16)
        make_identity(nc, ident_bf)
        iota_col = singles.tile([P, 1], f32)   # value = partition index
        nc.gpsimd.iota(
            iota_col,
            pattern=[[0, 1]],
            base=0,
            channel_multiplier=1,
            allow_small_or_imprecise_dtypes=True,
        )

        # ---- DVE constants
        scratch = singles.tile([P, BL], bf16)       # zeros: warmup weights+rhs
        nc.vector.memset(scratch, 0.0)
        invB_col = singles.tile([P, 1], bf16)
        nc.vector.memset(invB_col, 1.0 / B)
        negB_col = singles.tile([P, 1], bf16)
        nc.vector.memset(negB_col, -1.0 / B)
        sbigP = singles.tile([C + 1, BL], bf16)     # d2+1e9*(1-oh)+qpen
        # pad row C so transpose tiles are [101,128]; rows 96..99 are
        # overwritten by the mask STTs later (start partition must be 96)
        nc.vector.memset(sbigP[96 : C + 1, :], 1e9)

        # Sqrt table warmup: dependency-free ACT so the table load runs
        # during the DMA wait instead of gating the final sqrt.
        sqrt_warm = singles.tile([1, 1], f32)
        nc.scalar.activation(
            sqrt_warm, invB_col[0:1, :], mybir.ActivationFunctionType.Sqrt
        )

        # ---- PE warmup: zero-weight bf16 matmuls into the real PSUM
        # accumulation group.  They contribute exactly 0 and keep the PE
        # busy through the HAM window so the real matmuls run warm.
        s_ps = ps_big.tile([P, BL], f32)
        N_WARM = 7
        nc.tensor.matmul(
            s_ps, lhsT=scratch[:, :P], rhs=scratch, start=True, stop=False
        )
        for _ in range(N_WARM - 1):
            nc.tensor.matmul(
                s_ps, lhsT=scratch[:, :P], rhs=scratch, start=False, stop=False
            )

        # ---- one-hot masks (hidden under the DMA wait)
        tb = singles.tile([P, BL], bf16)
        nc.gpsimd.partition_broadcast(tb, trow)
        qpen_v = biga[:, 6 * P : 6 * P + 4].bitcast(f32)  # [P,1] f32 view
        ohT = singles.tile([C, BL], bf16)
        nc.vector.tensor_scalar(
            out=ohT, in0=tb[:C, :], scalar1=iota_col[:C, :], scalar2=0.0,
            op0=mybir.AluOpType.subtract, op1=mybir.AluOpType.is_equal,
        )
        maskM = singles.tile([C, BL], bf16)
        nc.vector.tensor_scalar(
            out=maskM, in0=ohT, scalar1=1e9, scalar2=qpen_v[:C, :],
            op0=mybir.AluOpType.mult, op1=mybir.AluOpType.add,
        )
        invOhT = singles.tile([C, BL], bf16)
        nc.vector.tensor_scalar(
            out=invOhT, in0=tb[:C, :], scalar1=iota_col[:C, :], scalar2=0.0,
            op0=mybir.AluOpType.subtract, op1=mybir.AluOpType.not_equal,
        )
        maskP = singles.tile([C, BL], bf16)
        nc.vector.tensor_scalar(
            out=maskP, in0=invOhT, scalar1=1e9, scalar2=qpen_v[:C, :],
            op0=mybir.AluOpType.mult, op1=mybir.AluOpType.add,
        )

        # ---- the real contraction: 3 dual-row fp8 matmuls
        DRSW = mybir.MatmulPerfMode.DoubleRowSwInterleave
        rhs_aps = [
            biga[:, 6 * P + 4 : 6 * P + 4 + 2 * BL],
            bigb[:, 0 : 2 * BL],
            bigb[:, 2 * BL : 4 * BL],
        ]
        for j in range(3):
            nc.tensor.matmul(
                s_ps,
                lhsT=biga[:, j * 2 * P : (j + 1) * 2 * P].bitcast(fp8),
                rhs=rhs_aps[j].bitcast(fp8).rearrange("p (k i) -> p k i", k=2),
                start=False, stop=(j == 2), perf_mode=DRSW,
            )

        # ---- masked squared matrices (PSUM -> bf16 halves, then mask STTs)
        d2q = singles.tile([C, BL], bf16)
        H = BL // 2
        for h in range(2):
            sl = slice(h * H, (h + 1) * H)
            nc.vector.tensor_copy(d2q[:, sl], s_ps[:C, sl])
            nc.vector.scalar_tensor_tensor(
                out=sbigM[:C, sl], in0=maskM[:, sl], scalar=1.0, in1=d2q[:, sl],
                op0=mybir.AluOpType.mult, op1=mybir.AluOpType.add,
            )
            nc.vector.scalar_tensor_tensor(
                out=sbigP[:C, sl], in0=maskP[:, sl], scalar=1.0, in1=d2q[:, sl],
                op0=mybir.AluOpType.mult, op1=mybir.AluOpType.add,
            )

        # ---- 8 transposes into two PSUM banks, 2 merged min-reduces
        CP = C + 2  # stride pad: 102*2B keeps PSUM writes 4B-aligned
        trM = ps_trm.tile([P, NT, CP], bf16)
        trP = ps_trp.tile([P, NT, CP], bf16)
        for t in range(NT):
            sl = slice(t * P, (t + 1) * P)
            nc.tensor.transpose(
                trM[:, t, : C + 1], sbigM[: C + 1, sl], ident_bf[: C + 1, : C + 1]
            )
            nc.tensor.transpose(
                trP[:, t, : C + 1], sbigP[: C + 1, sl], ident_bf[: C + 1, : C + 1]
            )
        mn8 = singles.tile([P, 2 * NT], f32)
        nc.vector.tensor_reduce(
            mn8[:, 0:NT], trM[:, :, : C + 1], axis=mybir.AxisListType.X,
            op=mybir.AluOpType.min,
        )
        nc.vector.tensor_reduce(
            mn8[:, NT : 2 * NT], trP[:, :, : C + 1], axis=mybir.AxisListType.X,
            op=mybir.AluOpType.min,
        )

        # ---- sqrt of the 8 per-tile columns, +-1/B matmuls, 4-elem reduce
        y_bf = singles.tile([P, 2 * NT], bf16)
        nc.scalar.activation(y_bf, mn8, mybir.ActivationFunctionType.Sqrt)
        fin_ps = ps_misc.tile([1, NT], f32, tag="misc")
        nc.tensor.matmul(
            fin_ps, lhsT=invB_col, rhs=y_bf[:, NT : 2 * NT], start=True, stop=False
        )
        nc.tensor.matmul(
            fin_ps, lhsT=negB_col, rhs=y_bf[:, 0:NT], start=False, stop=True
        )
        out_sb = singles.tile([1, 1], f32)
        nc.vector.tensor_reduce(
            out_sb, fin_ps, axis=mybir.AxisListType.X, op=mybir.AluOpType.add,
        )
        nc.sync.dma_start(out_ext, out_sb)

    nc.compile()
    return nc


def _get_nc():
    if "nc" not in _CACHE:
        _CACHE["nc"] = _build_nc()
    return _CACHE["nc"]


def make_in_maps(inputs, targets, centers, centers_weights):
    import ml_dtypes

    x = np.asarray(inputs, np.float32)
    t = np.asarray(targets).astype(np.int64)
    c = np.asarray(centers, np.float32)
    w = np.asarray(centers_weights, np.float32)
    assert x.shape == (B, D) and c.shape == (C, D) and w.shape == (C, D)

    fp8 = ml_dtypes.float8_e4m3
    bf16 = ml_dtypes.bfloat16

    xt_all = np.ascontiguousarray(x.astype(fp8).T)        # [D, B]
    x2t_all = np.ascontiguousarray((x * x).astype(fp8).T)  # [D, B]

    v = np.exp2(w)                                 # [C, D]
    u2 = -2.0 * v * c

    def pad_chunks(m_t):
        # [D, C] transposed weights, zero-padded to M=128, split into k-chunks
        m_p = np.zeros((D, P), fp8)
        m_p[:, :C] = m_t.astype(fp8)
        return m_p.reshape(KD, P, P)

    def ilv(a, b):
        # SwInterleave weight layout: pairwise interleave in reversed col order
        sw = np.empty((P, 2 * P), fp8)
        sw[:, 0::2] = a[:, ::-1]
        sw[:, 1::2] = b[:, ::-1]
        return sw

    uk = pad_chunks(u2.T)
    vk = pad_chunks(v.T)
    q = np.einsum("kd,kd->k", v, c * c).astype(np.float32)  # [C]
    present = np.bincount(t, minlength=C) > 0
    pen = np.where(present, 0.0, 1e12).astype(np.float32)
    qpen = np.zeros(P, np.float32)
    qpen[:C] = q + pen
    qpen_bytes = qpen.astype("<f4").reshape(P, 1).view(np.uint8)  # [P,4]

    ctr = np.ascontiguousarray(
        np.concatenate(
            [
                ilv(uk[0], uk[1]).view(np.uint8),
                ilv(vk[0], vk[1]).view(np.uint8),
                ilv(uk[2], vk[2]).view(np.uint8),
                qpen_bytes,
            ],
            axis=1,
        )
    )                                              # [P, 6P+4] raw bytes

    # closest-center distance (detached numpy constant in the original)
    diff = c[:, None, :] - c[None, :, :]           # [C, C, D]
    d2c = np.einsum("cd,ced->ce", v, diff * diff)
    dc = np.sqrt(d2c)
    dc.sort(axis=1)
    cd = dc[:, 1].astype(np.float32)               # [C]
    cc_all = cd[t]                                 # [B]
    cc2_all = (cc_all * cc_all).astype(np.float32)

    t_bf = t.astype(bf16)                          # targets < 100: exact in bf16

    pad = np.zeros((P, AW_PAD), np.uint8)

    in_maps = []
    for i in range(NUM_CORES):
        sl = slice(i * BL, (i + 1) * BL)
        xk = xt_all[:, sl].reshape(KD, P, BL)
        x2k = x2t_all[:, sl].reshape(KD, P, BL)
        p0 = np.stack([xk[0], xk[1]], axis=1).reshape(P, 2 * BL)
        p1 = np.stack([x2k[0], x2k[1]], axis=1).reshape(P, 2 * BL)
        p2 = np.stack([xk[2], x2k[2]], axis=1).reshape(P, 2 * BL)
        biga = np.concatenate([ctr, p0.view(np.uint8), pad], axis=1)
        bigb = np.concatenate([p1.view(np.uint8), p2.view(np.uint8)], axis=1)
        in_maps.append(
            {
                "biga": np.ascontiguousarray(biga),
                "bigb": np.ascontiguousarray(bigb),
                "trow": np.ascontiguousarray(t_bf[sl].reshape(1, BL)),
                "cc2": np.ascontiguousarray(cc2_all[sl].astype(bf16).reshape(1, BL)),
            }
        )
    return in_maps


AW_PAD = 2048 - (6 * P + 4 + 2 * BL)


def _host_cc_sum(targets, centers, centers_weights):
    t = np.asarray(targets).astype(np.int64)
    c = np.asarray(centers, np.float32)
    w = np.asarray(centers_weights, np.float32)
    v = np.exp2(w)
    diff = c[:, None, :] - c[None, :, :]
    d2c = np.einsum("cd,ced->ce", v, diff * diff)
    dc = np.sqrt(d2c)
    dc.sort(axis=1)
    cd = dc[:, 1].astype(np.float32)
    return float(np.sum(cd[t], dtype=np.float64))


def run(inputs, targets, centers, centers_weights, trace=False):
    """Build+run the SPMD kernel; returns (loss_scalar, BassKernelResults)."""
    from concourse import bass_utils

    nc = _get_nc()
    in_maps = make_in_maps(inputs, targets, centers, centers_weights)
    res = None
    for attempt in range(3):
        try:
            res = bass_utils.run_bass_kernel_spmd(
                nc, in_maps, core_ids=list(range(NUM_CORES)), trace=trace
            )
            break
        except Exception:
            # A previously-crashed session can leave the device in a transient
            # "unrecoverable" state that clears on the next attempt.
            if attempt == 2:
                raise
    loss = np.float32(0.0)
    for r in res.results:
        loss += np.float32(r["out"][0, 0])
    loss = np.float32(loss + _host_cc_sum(targets, centers, centers_weights) / B)
    return np.array(loss, dtype=np.float32), res


def kernel(inputs, targets, epoch_number=None, centers=None, centers_weights=None):
    loss, _ = run(inputs, targets, centers, centers_weights, trace=False)
    return loss


# revision 11
# speedup vs baseline: 1.1681x; 1.1681x over previous
"""Trainium2 Bass kernel for nn_AMTCL_77867757077077 (AMTCL triplet-center loss).

dist[i,j] depends on j only through targets[j] (C=100 distinct columns):
    d2[k,i] = q[k] - 2*(u[k]@x_i) + (v[k]@x2_i),  v=2^w, u=v*c, q=sum v*c^2
    per_sample = sqrt(d2[t_i,i]) + relu(cc_i - min_{k!=t_i} sqrt(d2[k,i]))

min and sqrt commute, and relu(cc-an) = cc - min(an, cc).  So per core:
    m2[i]  = min_k( d2 + qpen + 1e9*onehot ; row C = cc^2 )
    ap2[i] = min_k( d2 + qpen + 1e9*(1-onehot) )
    partial = sum_i ( sqrt(ap2) - sqrt(m2) ) / B
loss = sum_cores partial + sum_i cc_i / B   (host-side constant).
Only ONE [128,8] sqrt on device; everything else stays squared (bf16).

Streams (three parallel DMA queues):
    q10 (scalar HWDGE): wq [128,1024] = weights|qpen, then pairsH0 [128,1536]
    q1  (sync HWDGE):   pairsH1 [128,1536], then cc2 [1,512]
    SWDGE (gpsimd):     tbt [128,512] bf16 (targets broadcast to rows)
Column-halved x-pairs let the first 3 matmuls start ~0.9us earlier; the
masks (prebaked 1e9*oh+qpen via hidden DVE ops) are plain TT-adds from
PSUM.  Zero-weight bf16 warmup matmuls run during the DMA wait so the
real fp8 DoubleRowSwInterleave matmuls + transposes hit the 2.4GHz clock.
"""

import numpy as np

NUM_CORES = 8
B = 4096
D = 384
C = 100
BL = B // NUM_CORES  # 512 rows per core
P = 128
NT = BL // P         # 4 row tiles per core
KD = D // P          # 3 contraction chunks
H = BL // 2          # 256-column halves

_CACHE = {}


def _build_nc():
    import concourse.bass as bass
    import concourse.bass_isa as bass_isa
    import concourse.bacc as bacc
    import concourse.tile as tile
    from concourse import mybir
    from concourse.masks import make_identity
    from contextlib import ExitStack

    f32 = mybir.dt.float32
    bf16 = mybir.dt.bfloat16
    fp8 = mybir.dt.float8e4

    nc = bacc.Bacc(
        "TRN2",
        target_bir_lowering=False,
        debug=False,
        enable_asserts=False,
        num_devices=NUM_CORES,
    )

    wq_ext = nc.dram_tensor("wq", [P, 1024], mybir.dt.uint8, kind="ExternalInput").ap()
    ph0_ext = nc.dram_tensor("ph0", [P, 3 * BL], mybir.dt.uint8, kind="ExternalInput").ap()
    ph1_ext = nc.dram_tensor("ph1", [P, 3 * BL], mybir.dt.uint8, kind="ExternalInput").ap()
    tbt_ext = nc.dram_tensor("tbt", [P, BL], bf16, kind="ExternalInput").ap()
    cc2_ext = nc.dram_tensor("cc2", [1, BL], bf16, kind="ExternalInput").ap()
    out_ext = nc.dram_tensor("out", [1, 1], f32, kind="ExternalOutput").ap()

    with tile.TileContext(nc) as tc, ExitStack() as ctx:
        singles = ctx.enter_context(tc.tile_pool(name="singles", bufs=1))
        ps_big = ctx.enter_context(tc.tile_pool(name="psbig", bufs=1, space="PSUM"))
        ps_biga = ctx.enter_context(tc.tile_pool(name="psbiga", bufs=1, space="PSUM"))
        ps_wrm = ctx.enter_context(tc.tile_pool(name="pswrm", bufs=1, space="PSUM"))
        ps_trm = ctx.enter_context(tc.tile_pool(name="pstrm", bufs=1, space="PSUM"))
        ps_trp = ctx.enter_context(tc.tile_pool(name="pstrp", bufs=1, space="PSUM"))
        ps_misc = ctx.enter_context(tc.tile_pool(name="psmisc", bufs=1, space="PSUM"))

        # ---- input DMAs first on each queue
        wq = singles.tile([P, 1024], mybir.dt.uint8)
        nc.scalar.dma_start(wq, wq_ext)
        ph0 = singles.tile([P, 3 * BL], mybir.dt.uint8)
        nc.scalar.dma_start(ph0, ph0_ext)
        ph1 = singles.tile([P, 3 * BL], mybir.dt.uint8)
        nc.sync.dma_start(ph1, ph1_ext)
        sbigM = singles.tile([C + 1, BL], bf16)   # d2+qpen+1e9*oh; row C = cc^2
        nc.sync.dma_start(sbigM[C : C + 1, :], cc2_ext)
        tb = singles.tile([P, BL], bf16)          # targets broadcast (SWDGE)
        nc.gpsimd.dma_start(tb, tbt_ext)

        # ---- gpsimd constants (after the SWDGE issue; hidden)
        ident_bf = singles.tile([P, P], bf16)
        make_identity(nc, ident_bf)
        iota_col = singles.tile([P, 1], f32)   # value = partition index
        nc.gpsimd.iota(
            iota_col,
            pattern=[[0, 1]],
            base=0,
            channel_multiplier=1,
            allow_small_or_imprecise_dtypes=True,
        )

        # ---- DVE constants
        scratch = singles.tile([P, BL], bf16)       # zeros: warmup weights+rhs
        nc.vector.memset(scratch, 0.0)
        invB_col = singles.tile([P, 1], bf16)
        nc.vector.memset(invB_col, 1.0 / B)
        negB_col = singles.tile([P, 1], bf16)
        nc.vector.memset(negB_col, -1.0 / B)
        sbigP = singles.tile([C + 1, BL], bf16)     # d2+qpen+1e9*(1-oh)
        # pad row C big so transpose tiles are [101,128]; rows 96..99 are
        # overwritten by the mask TTs later (start partition must be 96)
        nc.vector.memset(sbigP[96 : C + 1, :], 1e9)

        # Sqrt table warmup: dependency-free ACT so the table load runs
        # during the DMA wait instead of gating the final sqrt.
        sqrt_warm = singles.tile([1, 1], f32)
        nc.scalar.activation(
            sqrt_warm, invB_col[0:1, :], mybir.ActivationFunctionType.Sqrt
        )

        # ---- PE warmup: zero matmuls into a scratch bank during DMA wait
        warm_ps = ps_wrm.tile([P, BL], f32)
        for _ in range(3):
            nc.tensor.matmul(
                warm_ps, lhsT=scratch[:, :P], rhs=scratch, start=True, stop=True
            )

        # ---- one-hot masks, prebaked with qpen (hidden under the DMA wait)
        qpen_v = wq[:, 6 * P : 6 * P + 4].bitcast(f32)  # [P,1] f32 view
        ohT = singles.tile([C, BL], bf16)
        nc.vector.tensor_scalar(
            out=ohT, in0=tb[:C, :], scalar1=iota_col[:C, :], scalar2=0.0,
            op0=mybir.AluOpType.subtract, op1=mybir.AluOpType.is_equal,
        )
        invOhT = singles.tile([C, BL], bf16)
        nc.vector.tensor_scalar(
            out=invOhT, in0=tb[:C, :], scalar1=iota_col[:C, :], scalar2=0.0,
            op0=mybir.AluOpType.subtract, op1=mybir.AluOpType.not_equal,
        )
        maskM = singles.tile([C, BL], bf16)
        nc.vector.tensor_scalar(
            out=maskM, in0=ohT, scalar1=1e9, scalar2=qpen_v[:C, :],
            op0=mybir.AluOpType.mult, op1=mybir.AluOpType.add,
        )
        maskP = singles.tile([C, BL], bf16)
        nc.vector.tensor_scalar(
            out=maskP, in0=invOhT, scalar1=1e9, scalar2=qpen_v[:C, :],
            op0=mybir.AluOpType.mult, op1=mybir.AluOpType.add,
        )

        # ---- 6 dual-row fp8 matmuls: 3 weight sets x 2 column halves.
        # Half B (cols 256:512, from q1/ph1) lands first; half A second.
        # Separate PSUM banks: interleaved accumulation groups cannot share
        # a zero region.
        s_psB = ps_big.tile([P, H], f32)
        s_psA = ps_biga.tile([P, H], f32)
        s_half = {0: s_psA, 1: s_psB}
        DRSW = mybir.MatmulPerfMode.DoubleRowSwInterleave

        def pair_rhs(t_, j):
            return (
                t_[:, j * 2 * H : (j + 1) * 2 * H]
                .bitcast(fp8)
                .rearrange("p (k i) -> p k i", k=2)
            )

        for j in range(3):
            wj = wq[:, j * 2 * P : (j + 1) * 2 * P].bitcast(fp8)
            nc.tensor.matmul(
                s_psB, lhsT=wj, rhs=pair_rhs(ph1, j),
                start=(j == 0), stop=(j == 2), perf_mode=DRSW,
            )
            nc.tensor.matmul(
                s_psA, lhsT=wj, rhs=pair_rhs(ph0, j),
                start=(j == 0), stop=(j == 2), perf_mode=DRSW,
            )

        # ---- masked squared matrices: plain TT adds (mask + PSUM)
        for h in (1, 0):  # half B first
            sl = slice(h * H, (h + 1) * H)
            nc.vector.tensor_add(sbigM[:C, sl], maskM[:, sl], s_half[h][:C, :])
            nc.vector.tensor_add(sbigP[:C, sl], maskP[:, sl], s_half[h][:C, :])

        # ---- 8 transposes into two PSUM banks, 2 merged min-reduces
        CP = C + 2  # stride pad: 102*2B keeps PSUM writes 4B-aligned
        trM = ps_trm.tile([P, NT, CP], bf16)
        trP = ps_trp.tile([P, NT, CP], bf16)
        for t in (2, 3, 0, 1):  # half-B column tiles first
            sl = slice(t * P, (t + 1) * P)
            nc.tensor.transpose(
                trM[:, t, : C + 1], sbigM[: C + 1, sl], ident_bf[: C + 1, : C + 1]
            )
            nc.tensor.transpose(
                trP[:, t, : C + 1], sbigP[: C + 1, sl], ident_bf[: C + 1, : C + 1]
            )
        mn8 = singles.tile([P, 2 * NT], f32)
        nc.vector.tensor_reduce(
            mn8[:, 0:NT], trM[:, :, : C + 1], axis=mybir.AxisListType.X,
            op=mybir.AluOpType.min,
        )
        nc.vector.tensor_reduce(
            mn8[:, NT : 2 * NT], trP[:, :, : C + 1], axis=mybir.AxisListType.X,
            op=mybir.AluOpType.min,
        )

        # ---- sqrt of the 8 per-tile columns, +-1/B matmuls, 4-elem reduce
        y_bf = singles.tile([P, 2 * NT], bf16)
        nc.scalar.activation(y_bf, mn8, mybir.ActivationFunctionType.Sqrt)
        fin_ps = ps_misc.tile([1, NT], f32, tag="misc")
        nc.tensor.matmul(
            fin_ps, lhsT=invB_col, rhs=y_bf[:, NT : 2 * NT], start=True, stop=False
        )
        nc.tensor.matmul(
            fin_ps, lhsT=negB_col, rhs=y_bf[:, 0:NT], start=False, stop=True
        )
        out_sb = singles.tile([1, 1], f32)
        nc.vector.tensor_reduce(
            out_sb, fin_ps, axis=mybir.AxisListType.X, op=mybir.AluOpType.add,
        )
        nc.sync.dma_start(out_ext, out_sb)

    nc.compile()
    return nc


def _get_nc():
    if "nc" not in _CACHE:
        _CACHE["nc"] = _build_nc()
    return _CACHE["nc"]


def make_in_maps(inputs, targets, centers, centers_weights):
    import ml_dtypes

    x = np.asarray(inputs, np.float32)
    t = np.asarray(targets).astype(np.int64)
    c = np.asarray(centers, np.float32)
    w = np.asarray(centers_weights, np.float32)
    assert x.shape == (B, D) and c.shape == (C, D) and w.shape == (C, D)

    fp8 = ml_dtypes.float8_e4m3
    bf16 = ml_dtypes.bfloat16

    xt_all = np.ascontiguousarray(x.astype(fp8).T)        # [D, B]
    x2t_all = np.ascontiguousarray((x * x).astype(fp8).T)  # [D, B]

    v = np.exp2(w)                                 # [C, D]
    u2 = -2.0 * v * c

    def pad_chunks(m_t):
        m_p = np.zeros((D, P), fp8)
        m_p[:, :C] = m_t.astype(fp8)
        return m_p.reshape(KD, P, P)

    def ilv(a, b):
        # SwInterleave weight layout: pairwise interleave in reversed col order
        sw = np.empty((P, 2 * P), fp8)
        sw[:, 0::2] = a[:, ::-1]
        sw[:, 1::2] = b[:, ::-1]
        return sw

    uk = pad_chunks(u2.T)
    vk = pad_chunks(v.T)
    q = np.einsum("kd,kd->k", v, c * c).astype(np.float32)  # [C]
    present = np.bincount(t, minlength=C) > 0
    pen = np.where(present, 0.0, 1e12).astype(np.float32)
    qpen = np.zeros(P, np.float32)
    qpen[:C] = q + pen
    qpen_bytes = qpen.astype("<f4").reshape(P, 1).view(np.uint8)  # [P,4]

    wq_t = np.zeros((P, 1024), np.uint8)
    wq_t[:, : 6 * P] = np.concatenate(
        [ilv(uk[0], uk[1]), ilv(vk[0], vk[1]), ilv(uk[2], vk[2])], axis=1
    ).view(np.uint8)
    wq_t[:, 6 * P : 6 * P + 4] = qpen_bytes

    # closest-center distance (detached numpy constant in the original)
    diff = c[:, None, :] - c[None, :, :]           # [C, C, D]
    d2c = np.einsum("cd,ced->ce", v, diff * diff)
    dc = np.sqrt(d2c)
    dc.sort(axis=1)
    cd = dc[:, 1].astype(np.float32)               # [C]
    cc_all = cd[t]                                 # [B]
    cc2_all = (cc_all * cc_all).astype(np.float32)

    t_bf = t.astype(bf16)                          # targets < 100: exact in bf16

    in_maps = []
    for i in range(NUM_CORES):
        sl = slice(i * BL, (i + 1) * BL)
        xk = xt_all[:, sl].reshape(KD, P, BL)
        x2k = x2t_all[:, sl].reshape(KD, P, BL)
        pairs = [(xk[0], xk[1]), (x2k[0], x2k[1]), (xk[2], x2k[2])]

        def half(hh):
            cs = slice(hh * H, (hh + 1) * H)
            return np.concatenate(
                [
                    np.stack([a[:, cs], b[:, cs]], axis=1).reshape(P, 2 * H)
                    for a, b in pairs
                ],
                axis=1,
            ).view(np.uint8)

        in_maps.append(
            {
                "wq": wq_t,
                "ph0": np.ascontiguousarray(half(0)),
                "ph1": np.ascontiguousarray(half(1)),
                "tbt": np.ascontiguousarray(
                    np.broadcast_to(t_bf[sl][None, :], (P, BL))
                ),
                "cc2": np.ascontiguousarray(cc2_all[sl].astype(bf16).reshape(1, BL)),
            }
        )
    return in_maps


def _host_cc_sum(targets, centers, centers_weights):
    t = np.asarray(targets).astype(np.int64)
    c = np.asarray(centers, np.float32)
    w = np.asarray(centers_weights, np.float32)
    v = np.exp2(w)
    diff = c[:, None, :] - c[None, :, :]
    d2c = np.einsum("cd,ced->ce", v, diff * diff)
    dc = np.sqrt(d2c)
    dc.sort(axis=1)
    cd = dc[:, 1].astype(np.float32)
    return float(np.sum(cd[t], dtype=np.float64))


def run(inputs, targets, centers, centers_weights, trace=False):
    """Build+run the SPMD kernel; returns (loss_scalar, BassKernelResults)."""
    from concourse import bass_utils

    nc = _get_nc()
    in_maps = make_in_maps(inputs, targets, centers, centers_weights)
    res = None
    for attempt in range(3):
        try:
            res = bass_utils.run_bass_kernel_spmd(
                nc, in_maps, core_ids=list(range(NUM_CORES)), trace=trace
            )
            break
        except Exception:
            if attempt == 2:
                raise
    loss = np.float32(0.0)
    for r in res.results:
        loss += np.float32(r["out"][0, 0])
    loss = np.float32(loss + _host_cc_sum(targets, centers, centers_weights) / B)
    return np.array(loss, dtype=np.float32), res


def kernel(inputs, targets, epoch_number=None, centers=None, centers_weights=None):
    loss, _ = run(inputs, targets, centers, centers_weights, trace=False)
    return loss


# revision 18
# speedup vs baseline: 1.1869x; 1.0161x over previous
"""Trainium2 Bass kernel for nn_AMTCL_77867757077077 (AMTCL triplet-center loss).

dist[i,j] depends on j only through targets[j] (C=100 distinct columns):
    d2[k,i] = q[k] - 2*(u[k]@x_i) + (v[k]@x2_i),  v=2^w, u=v*c, q=sum v*c^2
    per_sample = sqrt(d2[t_i,i]) + relu(cc_i - min_{k!=t_i} sqrt(d2[k,i]))

min and sqrt commute, and relu(cc-an) = cc - min(an, cc).  So per core:
    m2[i]  = min_k( d2 + qpen + 1e9*onehot ; row C = cc^2 )
    ap2[i] = min_k( d2 + qpen + 1e9*(1-onehot) )
    partial = sum_i ( sqrt(ap2) - sqrt(m2) ) / B
loss = sum_cores partial + sum_i cc_i / B   (host-side constant).
Only ONE [128,8] sqrt on device; everything else stays squared (bf16).

Streams (three parallel DMA queues):
    q10 (scalar HWDGE): wq [128,1024] = weights|qpen, then pairsH0 [128,1536]
    q1  (sync HWDGE):   pairsH1 [128,1536], then cc2 [1,512]
    SWDGE (gpsimd):     tbt [128,512] bf16 (targets broadcast to rows)
Column-halved x-pairs let the first 3 matmuls start ~0.9us earlier; the
masks (prebaked 1e9*oh+qpen via hidden DVE ops) are plain TT-adds from
PSUM.  Zero-weight bf16 warmup matmuls run during the DMA wait so the
real fp8 DoubleRowSwInterleave matmuls + transposes hit the 2.4GHz clock.
"""

import numpy as np

NUM_CORES = 8
B = 4096
D = 384
C = 100
BL = B // NUM_CORES  # 512 rows per core
P = 128
NT = BL // P         # 4 row tiles per core
KD = D // P          # 3 contraction chunks
H = BL // 2          # 256-column halves

_CACHE = {}


def _build_nc():
    import concourse.bass as bass
    import concourse.bass_isa as bass_isa
    import concourse.bacc as bacc
    import concourse.tile as tile
    from concourse import mybir
    from concourse.masks import make_identity
    from contextlib import ExitStack

    f32 = mybir.dt.float32
    bf16 = mybir.dt.bfloat16
    fp8 = mybir.dt.float8e4

    nc = bacc.Bacc(
        "TRN2",
        target_bir_lowering=False,
        debug=False,
        enable_asserts=False,
        num_devices=NUM_CORES,
    )

    wq_ext = nc.dram_tensor("wq", [P, 1024], mybir.dt.uint8, kind="ExternalInput").ap()
    ph0_ext = nc.dram_tensor("ph0", [P, 2 * BL], mybir.dt.uint8, kind="ExternalInput").ap()
    ph1_ext = nc.dram_tensor("ph1", [P, 4 * BL], mybir.dt.uint8, kind="ExternalInput").ap()
    trow_ext = nc.dram_tensor("trow", [1, BL], bf16, kind="ExternalInput").ap()
    cc2_ext = nc.dram_tensor("cc2", [1, BL], bf16, kind="ExternalInput").ap()
    out_ext = nc.dram_tensor("out", [1, 1], f32, kind="ExternalOutput").ap()

    with tile.TileContext(nc) as tc, ExitStack() as ctx:
        singles = ctx.enter_context(tc.tile_pool(name="singles", bufs=1))
        ps_big = ctx.enter_context(tc.tile_pool(name="psbig", bufs=1, space="PSUM"))
        ps_biga = ctx.enter_context(tc.tile_pool(name="psbiga", bufs=1, space="PSUM"))
        ps_wrm = ctx.enter_context(tc.tile_pool(name="pswrm", bufs=1, space="PSUM"))
        ps_tps = ctx.enter_context(tc.tile_pool(name="pstps", bufs=1, space="PSUM"))
        ps_trm = ctx.enter_context(tc.tile_pool(name="pstrm", bufs=1, space="PSUM"))
        ps_trp = ctx.enter_context(tc.tile_pool(name="pstrp", bufs=1, space="PSUM"))
        ps_misc = ctx.enter_context(tc.tile_pool(name="psmisc", bufs=1, space="PSUM"))

        # ---- input DMAs first on each queue.  q10 (scalar): weights, the
        # targets row, then H0 pairs 1-2.  q1 (sync): H1 pairs + H0 pair 0
        # (appended last), then cc^2.
        wq = singles.tile([P, 1024], mybir.dt.uint8)
        nc.scalar.dma_start(wq, wq_ext)
        trow = singles.tile([1, BL], bf16)
        nc.scalar.dma_start(trow, trow_ext)
        ph0 = singles.tile([P, 2 * BL], mybir.dt.uint8)
        nc.scalar.dma_start(ph0, ph0_ext)
        ph1 = singles.tile([P, 4 * BL], mybir.dt.uint8)
        nc.sync.dma_start(ph1, ph1_ext)
        sbigM = singles.tile([C + 1, BL], bf16)   # d2+qpen+1e9*oh; row C = cc^2
        nc.sync.dma_start(sbigM[C : C + 1, :], cc2_ext)

        # ---- gpsimd constants (after the SWDGE issue; hidden)
        ident_bf = singles.tile([P, P], bf16)
        make_identity(nc, ident_bf)
        iota_col = singles.tile([P, 1], f32)   # value = partition index
        nc.gpsimd.iota(
            iota_col,
            pattern=[[0, 1]],
            base=0,
            channel_multiplier=1,
            allow_small_or_imprecise_dtypes=True,
        )

        # ---- DVE constants
        scratch = singles.tile([P, BL], bf16)       # zeros: warmup weights+rhs
        nc.vector.memset(scratch, 0.0)
        ones_row = singles.tile([1, P], bf16)       # K=1 broadcast weights
        nc.vector.memset(ones_row, 1.0)
        invB_col = singles.tile([P, 1], bf16)
        nc.vector.memset(invB_col, 1.0 / B)
        negB_col = singles.tile([P, 1], bf16)
        nc.vector.memset(negB_col, -1.0 / B)
        sbigP = singles.tile([C + 1, BL], bf16)     # d2+qpen+1e9*(1-oh)
        # pad row C big so transpose tiles are [101,128]; rows 96..99 are
        # overwritten by the mask TTs later (start partition must be 96)
        nc.vector.memset(sbigP[96 : C + 1, :], 1e9)

        # Sqrt table warmup: dependency-free ACT so the table load runs
        # during the DMA wait instead of gating the final sqrt.
        sqrt_warm = singles.tile([1, 1], f32)
        nc.scalar.activation(
            sqrt_warm, invB_col[0:1, :], mybir.ActivationFunctionType.Sqrt
        )

        # ---- PE warmup: zero matmuls into a scratch bank during DMA wait,
        # with the K=1 targets-broadcast matmul slotted in once trow lands
        warm_ps = ps_wrm.tile([P, BL], f32)
        t_ps = ps_tps.tile([P, BL], f32)
        for _ in range(2):
            nc.tensor.matmul(
                warm_ps, lhsT=scratch[:, :P], rhs=scratch, start=True, stop=True
            )
        nc.tensor.matmul(t_ps, lhsT=ones_row, rhs=trow, start=True, stop=True)
        nc.tensor.matmul(
            warm_ps, lhsT=scratch[:, :P], rhs=scratch, start=True, stop=True
        )

        # ---- one-hot masks, prebaked with qpen (hidden under the DMA wait)
        qpen_v = wq[:, 6 * P : 6 * P + 4].bitcast(f32)  # [P,1] f32 view
        ohT = singles.tile([C, BL], bf16)
        nc.vector.tensor_scalar(
            out=ohT, in0=t_ps[:C, :], scalar1=iota_col[:C, :], scalar2=0.0,
            op0=mybir.AluOpType.subtract, op1=mybir.AluOpType.is_equal,
        )
        invOhT = singles.tile([C, BL], bf16)
        nc.vector.tensor_scalar(
            out=invOhT, in0=t_ps[:C, :], scalar1=iota_col[:C, :], scalar2=0.0,
            op0=mybir.AluOpType.subtract, op1=mybir.AluOpType.not_equal,
        )
        maskM = singles.tile([C, BL], bf16)
        nc.vector.tensor_scalar(
            out=maskM, in0=ohT, scalar1=1e9, scalar2=qpen_v[:C, :],
            op0=mybir.AluOpType.mult, op1=mybir.AluOpType.add,
        )
        maskP = singles.tile([C, BL], bf16)
        nc.vector.tensor_scalar(
            out=maskP, in0=invOhT, scalar1=1e9, scalar2=qpen_v[:C, :],
            op0=mybir.AluOpType.mult, op1=mybir.AluOpType.add,
        )

        # ---- 6 dual-row fp8 matmuls: 3 weight sets x 2 column halves.
        # Half B (cols 256:512, from q1/ph1) lands first; half A second.
        # Separate PSUM banks: interleaved accumulation groups cannot share
        # a zero region.
        s_psB = ps_big.tile([P, H], f32)
        s_psA = ps_biga.tile([P, H], f32)
        s_half = {0: s_psA, 1: s_psB}
        DRSW = mybir.MatmulPerfMode.DoubleRowSwInterleave

        def pair_rhs(t_, off):
            return (
                t_[:, off : off + 2 * H]
                .bitcast(fp8)
                .rearrange("p (k i) -> p k i", k=2)
            )

        # A-half pair j lives at: j=0 -> ph1[1536:], j=1 -> ph0[0:], j=2 -> ph0[512:]
        a_src = [(ph1, 3 * 2 * H), (ph0, 0), (ph0, 2 * H)]
        for j in range(3):
            wj = wq[:, j * 2 * P : (j + 1) * 2 * P].bitcast(fp8)
            nc.tensor.matmul(
                s_psB, lhsT=wj, rhs=pair_rhs(ph1, j * 2 * H),
                start=(j == 0), stop=(j == 2), perf_mode=DRSW,
            )
            nc.tensor.matmul(
                s_psA, lhsT=wj, rhs=pair_rhs(*a_src[j]),
                start=(j == 0), stop=(j == 2), perf_mode=DRSW,
            )

        # ---- masked squared matrices: plain TT adds (mask + PSUM)
        for h in (1, 0):  # half B first
            sl = slice(h * H, (h + 1) * H)
            nc.vector.tensor_add(sbigM[:C, sl], maskM[:, sl], s_half[h][:C, :])
            nc.vector.tensor_add(sbigP[:C, sl], maskP[:, sl], s_half[h][:C, :])

        # ---- 8 transposes into two PSUM banks, 2 merged min-reduces
        CP = C + 2  # stride pad: 102*2B keeps PSUM writes 4B-aligned
        trM = ps_trm.tile([P, NT, CP], bf16)
        trP = ps_trp.tile([P, NT, CP], bf16)
        for t in (2, 3, 0, 1):  # half-B column tiles first
            sl = slice(t * P, (t + 1) * P)
            nc.tensor.transpose(
                trM[:, t, : C + 1], sbigM[: C + 1, sl], ident_bf[: C + 1, : C + 1]
            )
            nc.tensor.transpose(
                trP[:, t, : C + 1], sbigP[: C + 1, sl], ident_bf[: C + 1, : C + 1]
            )
        mn8 = singles.tile([P, 2 * NT], f32)
        nc.vector.tensor_reduce(
            mn8[:, 0:NT], trM[:, :, : C + 1], axis=mybir.AxisListType.X,
            op=mybir.AluOpType.min,
        )
        nc.vector.tensor_reduce(
            mn8[:, NT : 2 * NT], trP[:, :, : C + 1], axis=mybir.AxisListType.X,
            op=mybir.AluOpType.min,
        )

        # ---- sqrt of the 8 per-tile columns, +-1/B matmuls, 4-elem reduce
        y_bf = singles.tile([P, 2 * NT], bf16)
        nc.scalar.activation(y_bf, mn8, mybir.ActivationFunctionType.Sqrt)
        fin_ps = ps_misc.tile([1, NT], f32, tag="misc")
        nc.tensor.matmul(
            fin_ps, lhsT=invB_col, rhs=y_bf[:, NT : 2 * NT], start=True, stop=False
        )
        nc.tensor.matmul(
            fin_ps, lhsT=negB_col, rhs=y_bf[:, 0:NT], start=False, stop=True
        )
        out_sb = singles.tile([1, 1], f32)
        nc.vector.tensor_reduce(
            out_sb, fin_ps, axis=mybir.AxisListType.X, op=mybir.AluOpType.add,
        )
        nc.sync.dma_start(out_ext, out_sb)

    nc.compile()
    return nc


def _get_nc():
    if "nc" not in _CACHE:
        _CACHE["nc"] = _build_nc()
    return _CACHE["nc"]


def make_in_maps(inputs, targets, centers, centers_weights):
    import ml_dtypes

    x = np.asarray(inputs, np.float32)
    t = np.asarray(targets).astype(np.int64)
    c = np.asarray(centers, np.float32)
    w = np.asarray(centers_weights, np.float32)
    assert x.shape == (B, D) and c.shape == (C, D) and w.shape == (C, D)

    fp8 = ml_dtypes.float8_e4m3
    bf16 = ml_dtypes.bfloat16

    xt_all = np.ascontiguousarray(x.astype(fp8).T)        # [D, B]
    x2t_all = np.ascontiguousarray((x * x).astype(fp8).T)  # [D, B]

    v = np.exp2(w)                                 # [C, D]
    u2 = -2.0 * v * c

    def pad_chunks(m_t):
        m_p = np.zeros((D, P), fp8)
        m_p[:, :C] = m_t.astype(fp8)
        return m_p.reshape(KD, P, P)

    def ilv(a, b):
        # SwInterleave weight layout: pairwise interleave in reversed col order
        sw = np.empty((P, 2 * P), fp8)
        sw[:, 0::2] = a[:, ::-1]
        sw[:, 1::2] = b[:, ::-1]
        return sw

    uk = pad_chunks(u2.T)
    vk = pad_chunks(v.T)
    q = np.einsum("kd,kd->k", v, c * c).astype(np.float32)  # [C]
    present = np.bincount(t, minlength=C) > 0
    pen = np.where(present, 0.0, 1e12).astype(np.float32)
    qpen = np.zeros(P, np.float32)
    qpen[:C] = q + pen
    qpen_bytes = qpen.astype("<f4").reshape(P, 1).view(np.uint8)  # [P,4]

    wq_t = np.zeros((P, 1024), np.uint8)
    wq_t[:, : 6 * P] = np.concatenate(
        [ilv(uk[0], uk[1]), ilv(vk[0], vk[1]), ilv(uk[2], vk[2])], axis=1
    ).view(np.uint8)
    wq_t[:, 6 * P : 6 * P + 4] = qpen_bytes

    # closest-center distance (detached numpy constant in the original)
    diff = c[:, None, :] - c[None, :, :]           # [C, C, D]
    d2c = np.einsum("cd,ced->ce", v, diff * diff)
    dc = np.sqrt(d2c)
    dc.sort(axis=1)
    cd = dc[:, 1].astype(np.float32)               # [C]
    cc_all = cd[t]                                 # [B]
    cc2_all = (cc_all * cc_all).astype(np.float32)

    t_bf = t.astype(bf16)                          # targets < 100: exact in bf16

    in_maps = []
    for i in range(NUM_CORES):
        sl = slice(i * BL, (i + 1) * BL)
        xk = xt_all[:, sl].reshape(KD, P, BL)
        x2k = x2t_all[:, sl].reshape(KD, P, BL)
        pairs = [(xk[0], xk[1]), (x2k[0], x2k[1]), (xk[2], x2k[2])]

        def pair_half(j, hh):
            cs = slice(hh * H, (hh + 1) * H)
            a, b = pairs[j]
            return np.stack([a[:, cs], b[:, cs]], axis=1).reshape(P, 2 * H)

        ph1_t = np.concatenate(
            [pair_half(0, 1), pair_half(1, 1), pair_half(2, 1), pair_half(0, 0)],
            axis=1,
        ).view(np.uint8)
        ph0_t = np.concatenate([pair_half(1, 0), pair_half(2, 0)], axis=1).view(
            np.uint8
        )
        in_maps.append(
            {
                "wq": wq_t,
                "ph0": np.ascontiguousarray(ph0_t),
                "ph1": np.ascontiguousarray(ph1_t),
                "trow": np.ascontiguousarray(t_bf[sl].reshape(1, BL)),
                "cc2": np.ascontiguousarray(cc2_all[sl].astype(bf16).reshape(1, BL)),
            }
        )
    return in_maps


def _host_cc_sum(targets, centers, centers_weights):
    t = np.asarray(targets).astype(np.int64)
    c = np.asarray(centers, np.float32)
    w = np.asarray(centers_weights, np.float32)
    v = np.exp2(w)
    diff = c[:, None, :] - c[None, :, :]
    d2c = np.einsum("cd,ced->ce", v, diff * diff)
    dc = np.sqrt(d2c)
    dc.sort(axis=1)
    cd = dc[:, 1].astype(np.float32)
    return float(np.sum(cd[t], dtype=np.float64))


def run(inputs, targets, centers, centers_weights, trace=False):
    """Build+run the SPMD kernel; returns (loss_scalar, BassKernelResults)."""
    from concourse import bass_utils

    nc = _get_nc()
    in_maps = make_in_maps(inputs, targets, centers, centers_weights)
    res = None
    for attempt in range(3):
        try:
            res = bass_utils.run_bass_kernel_spmd(
                nc, in_maps, core_ids=list(range(NUM_CORES)), trace=trace
            )
            break
        except Exception:
            if attempt == 2:
                raise
    loss = np.float32(0.0)
    for r in res.results:
        loss += np.float32(r["out"][0, 0])
    loss = np.float32(loss + _host_cc_sum(targets, centers, centers_weights) / B)
    return np.array(loss, dtype=np.float32), res


def kernel(inputs, targets, epoch_number=None, centers=None, centers_weights=None):
    loss, _ = run(inputs, targets, centers, centers_weights, trace=False)
    return loss


# revision 21
# speedup vs baseline: 1.2723x; 1.0720x over previous
"""Trainium2 Bass kernel for nn_AMTCL_77867757077077 (AMTCL triplet-center loss).

dist[i,j] depends on j only through targets[j] (C=100 distinct columns):
    d2[k,i] = q[k] - 2*(u[k]@x_i) + (v[k]@x2_i),  v=2^w, u=v*c, q=sum v*c^2
    per_sample = sqrt(d2[t_i,i]) + relu(cc_i - min_{k!=t_i} sqrt(d2[k,i]))

min and sqrt commute, and relu(cc-an) = cc - min(an, cc).  So per core:
    m2[i]  = min_k( d2 + qpen + 1e9*onehot ; row C = cc^2 )
    ap2[i] = min_k( d2 + qpen + 1e9*(1-onehot) )
    partial = sum_i ( sqrt(ap2) - sqrt(m2) ) / B
loss = sum_cores partial + sum_i cc_i / B   (host-side constant).
Only ONE [128,8] sqrt on device; everything else stays squared (bf16).

Streams (three parallel DMA queues):
    q10 (scalar HWDGE): wq [128,1024] = weights|qpen, then pairsH0 [128,1536]
    q1  (sync HWDGE):   pairsH1 [128,1536], then cc2 [1,512]
    SWDGE (gpsimd):     tbt [128,512] bf16 (targets broadcast to rows)
Column-halved x-pairs let the first 3 matmuls start ~0.9us earlier; the
masks (prebaked 1e9*oh+qpen via hidden DVE ops) are plain TT-adds from
PSUM.  Zero-weight bf16 warmup matmuls run during the DMA wait so the
real fp8 DoubleRowSwInterleave matmuls + transposes hit the 2.4GHz clock.
"""

import numpy as np

NUM_CORES = 8
B = 4096
D = 384
C = 100
BL = B // NUM_CORES  # 512 rows per core
P = 128
NT = BL // P         # 4 row tiles per core
KD = D // P          # 3 contraction chunks
H = BL // 2          # 256-column halves

_CACHE = {}


def _build_nc():
    import concourse.bass as bass
    import concourse.bass_isa as bass_isa
    import concourse.bacc as bacc
    import concourse.tile as tile
    from concourse import mybir
    from concourse.masks import make_identity
    from contextlib import ExitStack

    f32 = mybir.dt.float32
    bf16 = mybir.dt.bfloat16
    fp8 = mybir.dt.float8e4

    nc = bacc.Bacc(
        "TRN2",
        target_bir_lowering=False,
        debug=False,
        enable_asserts=False,
        num_devices=NUM_CORES,
    )

    wq_ext = nc.dram_tensor("wq", [P, 1024], mybir.dt.uint8, kind="ExternalInput").ap()
    ph0_ext = nc.dram_tensor("ph0", [P, 2 * BL], mybir.dt.uint8, kind="ExternalInput").ap()
    ph1_ext = nc.dram_tensor("ph1", [P, 4 * BL], mybir.dt.uint8, kind="ExternalInput").ap()
    trow_ext = nc.dram_tensor("trow", [1, BL], bf16, kind="ExternalInput").ap()
    cc2_ext = nc.dram_tensor("cc2", [1, BL], bf16, kind="ExternalInput").ap()
    out_ext = nc.dram_tensor("out", [1, 1], f32, kind="ExternalOutput").ap()

    with tile.TileContext(nc) as tc, ExitStack() as ctx:
        singles = ctx.enter_context(tc.tile_pool(name="singles", bufs=1))
        ps_big = ctx.enter_context(tc.tile_pool(name="psbig", bufs=1, space="PSUM"))
        ps_biga = ctx.enter_context(tc.tile_pool(name="psbiga", bufs=1, space="PSUM"))
        ps_wrm = ctx.enter_context(tc.tile_pool(name="pswrm", bufs=1, space="PSUM"))
        ps_tps = ctx.enter_context(tc.tile_pool(name="pstps", bufs=1, space="PSUM"))
        ps_trm = ctx.enter_context(tc.tile_pool(name="pstrm", bufs=1, space="PSUM"))
        ps_trp = ctx.enter_context(tc.tile_pool(name="pstrp", bufs=1, space="PSUM"))
        ps_misc = ctx.enter_context(tc.tile_pool(name="psmisc", bufs=1, space="PSUM"))

        # ---- input DMAs first on each queue.  q10 (scalar): weights, the
        # targets row, then H0 pairs 1-2.  q1 (sync): H1 pairs + H0 pair 0
        # (appended last), then cc^2.
        trow = singles.tile([1, BL], bf16)
        nc.scalar.dma_start(trow, trow_ext)
        wq = singles.tile([P, 1024], mybir.dt.uint8)
        nc.scalar.dma_start(wq, wq_ext)
        ph0 = singles.tile([P, 2 * BL], mybir.dt.uint8)
        nc.scalar.dma_start(ph0, ph0_ext)
        ph1 = singles.tile([P, 4 * BL], mybir.dt.uint8)
        nc.sync.dma_start(ph1, ph1_ext)
        sbigM = singles.tile([C + 1, BL], bf16)   # d2+qpen+1e9*oh; row C = cc^2
        nc.sync.dma_start(sbigM[C : C + 1, :], cc2_ext)

        # ---- gpsimd constants (after the SWDGE issue; hidden)
        ident_bf = singles.tile([P, P], bf16)
        make_identity(nc, ident_bf)
        iota_col = singles.tile([P, 1], f32)   # value = partition index
        nc.gpsimd.iota(
            iota_col,
            pattern=[[0, 1]],
            base=0,
            channel_multiplier=1,
            allow_small_or_imprecise_dtypes=True,
        )

        # ---- DVE constants
        scratch = singles.tile([P, BL], bf16)       # zeros: warmup weights+rhs
        nc.vector.memset(scratch, 0.0)
        ones_row = singles.tile([1, P], bf16)       # K=1 broadcast weights
        nc.vector.memset(ones_row, 1.0)
        invB_col = singles.tile([P, 1], bf16)
        nc.vector.memset(invB_col, 1.0 / B)
        negB_col = singles.tile([P, 1], bf16)
        nc.vector.memset(negB_col, -1.0 / B)
        sbigP = singles.tile([C + 1, BL], bf16)     # d2+qpen+1e9*(1-oh)
        # pad row C big so transpose tiles are [101,128]; rows 96..99 are
        # overwritten by the mask TTs later (start partition must be 96)
        nc.vector.memset(sbigP[96 : C + 1, :], 1e9)

        # Sqrt table warmup: dependency-free ACT so the table load runs
        # during the DMA wait instead of gating the final sqrt.
        sqrt_warm = singles.tile([1, 1], f32)
        nc.scalar.activation(
            sqrt_warm, invB_col[0:1, :], mybir.ActivationFunctionType.Sqrt
        )

        # ---- PE warmup: zero matmuls into a scratch bank during DMA wait,
        # with the K=1 targets-broadcast matmul slotted in once trow lands
        warm_ps = ps_wrm.tile([P, BL], f32)
        t_ps = ps_tps.tile([P, BL], f32)
        for _ in range(2):
            nc.tensor.matmul(
                warm_ps, lhsT=scratch[:, :P], rhs=scratch, start=True, stop=True
            )
        nc.tensor.matmul(t_ps, lhsT=ones_row, rhs=trow, start=True, stop=True)
        for _ in range(3):
            nc.tensor.matmul(
                warm_ps, lhsT=scratch[:, :P], rhs=scratch, start=True, stop=True
            )

        # ---- one-hot masks, prebaked with qpen (hidden under the DMA wait)
        qpen_v = wq[:, 6 * P : 6 * P + 4].bitcast(f32)  # [P,1] f32 view
        ohT = singles.tile([C, BL], bf16)
        nc.vector.tensor_scalar(
            out=ohT, in0=t_ps[:C, :], scalar1=iota_col[:C, :], scalar2=0.0,
            op0=mybir.AluOpType.subtract, op1=mybir.AluOpType.is_equal,
        )
        invOhT = singles.tile([C, BL], bf16)
        nc.vector.tensor_scalar(
            out=invOhT, in0=t_ps[:C, :], scalar1=iota_col[:C, :], scalar2=0.0,
            op0=mybir.AluOpType.subtract, op1=mybir.AluOpType.not_equal,
        )
        maskM = singles.tile([C, BL], bf16)
        nc.vector.tensor_scalar(
            out=maskM, in0=ohT, scalar1=1e9, scalar2=qpen_v[:C, :],
            op0=mybir.AluOpType.mult, op1=mybir.AluOpType.add,
        )
        maskP = singles.tile([C, BL], bf16)
        nc.vector.tensor_scalar(
            out=maskP, in0=invOhT, scalar1=1e9, scalar2=qpen_v[:C, :],
            op0=mybir.AluOpType.mult, op1=mybir.AluOpType.add,
        )

        # ---- 6 dual-row fp8 matmuls: 3 weight sets x 2 column halves.
        # Half B (cols 256:512, from q1/ph1) lands first; half A second.
        # Separate PSUM banks: interleaved accumulation groups cannot share
        # a zero region.
        s_psB = ps_big.tile([P, H], f32)
        s_psA = ps_biga.tile([P, H], f32)
        s_half = {0: s_psA, 1: s_psB}
        DRSW = mybir.MatmulPerfMode.DoubleRowSwInterleave

        def pair_rhs(t_, off):
            return (
                t_[:, off : off + 2 * H]
                .bitcast(fp8)
                .rearrange("p (k i) -> p k i", k=2)
            )

        # A-half pair j lives at: j=0 -> ph1[1536:], j=1 -> ph0[0:], j=2 -> ph0[512:]
        a_src = [(ph1, 3 * 2 * H), (ph0, 0), (ph0, 2 * H)]
        for j in range(3):
            wj = wq[:, j * 2 * P : (j + 1) * 2 * P].bitcast(fp8)
            nc.tensor.matmul(
                s_psB, lhsT=wj, rhs=pair_rhs(ph1, j * 2 * H),
                start=(j == 0), stop=(j == 2), perf_mode=DRSW,
            )
            nc.tensor.matmul(
                s_psA, lhsT=wj, rhs=pair_rhs(*a_src[j]),
                start=(j == 0), stop=(j == 2), perf_mode=DRSW,
            )

        # ---- masked squared matrices: plain TT adds (mask + PSUM)
        for h in (1, 0):  # half B first
            sl = slice(h * H, (h + 1) * H)
            nc.vector.tensor_add(sbigM[:C, sl], maskM[:, sl], s_half[h][:C, :])
            nc.vector.tensor_add(sbigP[:C, sl], maskP[:, sl], s_half[h][:C, :])

        # ---- 8 transposes into two PSUM banks, 2 merged min-reduces
        CP = C + 2  # stride pad: 102*2B keeps PSUM writes 4B-aligned
        trM = ps_trm.tile([P, NT, CP], bf16)
        trP = ps_trp.tile([P, NT, CP], bf16)
        for t in (2, 3, 0, 1):  # half-B column tiles first
            sl = slice(t * P, (t + 1) * P)
            nc.tensor.transpose(
                trM[:, t, : C + 1], sbigM[: C + 1, sl], ident_bf[: C + 1, : C + 1]
            )
            nc.tensor.transpose(
                trP[:, t, : C + 1], sbigP[: C + 1, sl], ident_bf[: C + 1, : C + 1]
            )
        # half-split reduces so half B's tail overlaps half A's transposes
        mn8 = singles.tile([P, 2 * NT], f32)
        for lo, hi in ((2, 4), (0, 2)):  # tiles 2,3 (half B) first
            nc.vector.tensor_reduce(
                mn8[:, lo:hi], trM[:, lo:hi, : C + 1], axis=mybir.AxisListType.X,
                op=mybir.AluOpType.min,
            )
            nc.vector.tensor_reduce(
                mn8[:, NT + lo : NT + hi], trP[:, lo:hi, : C + 1],
                axis=mybir.AxisListType.X, op=mybir.AluOpType.min,
            )

        # ---- sqrt of the 8 per-tile columns, +-1/B matmuls, 4-elem reduce
        y_bf = singles.tile([P, 2 * NT], bf16)
        nc.scalar.activation(y_bf, mn8, mybir.ActivationFunctionType.Sqrt)
        fin_ps = ps_misc.tile([1, NT], f32, tag="misc")
        nc.tensor.matmul(
            fin_ps, lhsT=invB_col, rhs=y_bf[:, NT : 2 * NT], start=True, stop=False
        )
        nc.tensor.matmul(
            fin_ps, lhsT=negB_col, rhs=y_bf[:, 0:NT], start=False, stop=True
        )
        out_sb = singles.tile([1, 1], f32)
        nc.vector.tensor_reduce(
            out_sb, fin_ps, axis=mybir.AxisListType.X, op=mybir.AluOpType.add,
        )
        nc.sync.dma_start(out_ext, out_sb)

    nc.compile()
    return nc


def _get_nc():
    if "nc" not in _CACHE:
        _CACHE["nc"] = _build_nc()
    return _CACHE["nc"]


def make_in_maps(inputs, targets, centers, centers_weights):
    import ml_dtypes

    x = np.asarray(inputs, np.float32)
    t = np.asarray(targets).astype(np.int64)
    c = np.asarray(centers, np.float32)
    w = np.asarray(centers_weights, np.float32)
    assert x.shape == (B, D) and c.shape == (C, D) and w.shape == (C, D)

    fp8 = ml_dtypes.float8_e4m3
    bf16 = ml_dtypes.bfloat16

    xt_all = np.ascontiguousarray(x.astype(fp8).T)        # [D, B]
    x2t_all = np.ascontiguousarray((x * x).astype(fp8).T)  # [D, B]

    v = np.exp2(w)                                 # [C, D]
    u2 = -2.0 * v * c

    def pad_chunks(m_t):
        m_p = np.zeros((D, P), fp8)
        m_p[:, :C] = m_t.astype(fp8)
        return m_p.reshape(KD, P, P)

    def ilv(a, b):
        # SwInterleave weight layout: pairwise interleave in reversed col order
        sw = np.empty((P, 2 * P), fp8)
        sw[:, 0::2] = a[:, ::-1]
        sw[:, 1::2] = b[:, ::-1]
        return sw

    uk = pad_chunks(u2.T)
    vk = pad_chunks(v.T)
    q = np.einsum("kd,kd->k", v, c * c).astype(np.float32)  # [C]
    present = np.bincount(t, minlength=C) > 0
    pen = np.where(present, 0.0, 1e12).astype(np.float32)
    qpen = np.zeros(P, np.float32)
    qpen[:C] = q + pen
    qpen_bytes = qpen.astype("<f4").reshape(P, 1).view(np.uint8)  # [P,4]

    wq_t = np.zeros((P, 1024), np.uint8)
    wq_t[:, : 6 * P] = np.concatenate(
        [ilv(uk[0], uk[1]), ilv(vk[0], vk[1]), ilv(uk[2], vk[2])], axis=1
    ).view(np.uint8)
    wq_t[:, 6 * P : 6 * P + 4] = qpen_bytes

    # closest-center distance (detached numpy constant in the original)
    diff = c[:, None, :] - c[None, :, :]           # [C, C, D]
    d2c = np.einsum("cd,ced->ce", v, diff * diff)
    dc = np.sqrt(d2c)
    dc.sort(axis=1)
    cd = dc[:, 1].astype(np.float32)               # [C]
    cc_all = cd[t]                                 # [B]
    cc2_all = (cc_all * cc_all).astype(np.float32)

    t_bf = t.astype(bf16)                          # targets < 100: exact in bf16

    in_maps = []
    for i in range(NUM_CORES):
        sl = slice(i * BL, (i + 1) * BL)
        xk = xt_all[:, sl].reshape(KD, P, BL)
        x2k = x2t_all[:, sl].reshape(KD, P, BL)
        pairs = [(xk[0], xk[1]), (x2k[0], x2k[1]), (xk[2], x2k[2])]

        def pair_half(j, hh):
            cs = slice(hh * H, (hh + 1) * H)
            a, b = pairs[j]
            return np.stack([a[:, cs], b[:, cs]], axis=1).reshape(P, 2 * H)

        ph1_t = np.concatenate(
            [pair_half(0, 1), pair_half(1, 1), pair_half(2, 1), pair_half(0, 0)],
            axis=1,
        ).view(np.uint8)
        ph0_t = np.concatenate([pair_half(1, 0), pair_half(2, 0)], axis=1).view(
            np.uint8
        )
        in_maps.append(
            {
                "wq": wq_t,
                "ph0": np.ascontiguousarray(ph0_t),
                "ph1": np.ascontiguousarray(ph1_t),
                "trow": np.ascontiguousarray(t_bf[sl].reshape(1, BL)),
                "cc2": np.ascontiguousarray(cc2_all[sl].astype(bf16).reshape(1, BL)),
            }
        )
    return in_maps


def _host_cc_sum(targets, centers, centers_weights):
    t = np.asarray(targets).astype(np.int64)
    c = np.asarray(centers, np.float32)
    w = np.asarray(centers_weights, np.float32)
    v = np.exp2(w)
    diff = c[:, None, :] - c[None, :, :]
    d2c = np.einsum("cd,ced->ce", v, diff * diff)
    dc = np.sqrt(d2c)
    dc.sort(axis=1)
    cd = dc[:, 1].astype(np.float32)
    return float(np.sum(cd[t], dtype=np.float64))


def run(inputs, targets, centers, centers_weights, trace=False):
    """Build+run the SPMD kernel; returns (loss_scalar, BassKernelResults)."""
    from concourse import bass_utils

    nc = _get_nc()
    in_maps = make_in_maps(inputs, targets, centers, centers_weights)
    res = None
    for attempt in range(3):
        try:
            res = bass_utils.run_bass_kernel_spmd(
                nc, in_maps, core_ids=list(range(NUM_CORES)), trace=trace
            )
            break
        except Exception:
            if attempt == 2:
                raise
    loss = np.float32(0.0)
    for r in res.results:
        loss += np.float32(r["out"][0, 0])
    loss = np.float32(loss + _host_cc_sum(targets, centers, centers_weights) / B)
    return np.array(loss, dtype=np.float32), res


def kernel(inputs, targets, epoch_number=None, centers=None, centers_weights=None):
    loss, _ = run(inputs, targets, centers, centers_weights, trace=False)
    return loss
